# revision 51
# baseline (speedup 1.0000x reference)
"""Bahdanau-attention Trainium2 kernel (data-parallel over 8 NeuronCores).

Computation (per batch row b):
    energy[s, d] = tanh(hidden[b] @ W_h + enc[b, s] @ W_e + b_attn)   [S, D]
    scores[s]    = energy[s] . w_v                                     [S]
    attn         = softmax(scores)                                     [S]
    out[b]       = sum_s attn[s] * enc[b, s]                           [E]

v10 — fp8 DoubleRow energy, v8 score structure, tuned pipeline:
  - h_proj precomputed host-side; biasT [P, DC, BL] fp32 input.
  - energy matmuls in fp8e4m3 with perf_mode=DoubleRow (K=256 per MM):
    enc staged [BL, NST, P, G, KO, ST], W_e [G, KO, P, DEC].
  - scores: tanh -> x w_v (DVE tensor_scalar) -> running-sum adds (DVE,
    keeps the post-dc3 chain to one add) -> 8 single-shot column
    matmuls (asum slice stationary x ones) -> exp into probsT.
  - per-half emission order spreads the previous half's weighted-sum
    burst (dependency-free PE filler) through the energy stream:
      E0 E1 W[0:2] E2 W[2:4] E3 W[4:8] S exp
    covering both the psum-recycle waits (E_i needs tanh_{i-2} done,
    ps_e has 2 bufs) and the tanh3->mul->add chain ahead of S.
  - softmax sums/recip emitted BEFORE the final weighted-sum burst so
    the output scale is ready when the orow accumulation stops; output
    row scaling on DVE (tensor_scalar), off the ACT engine.
  - single-dma_start bulk transfers (~600ns issue each), 2-8KB
    contiguous per partition row; batch 0's encT blocks split across
    both HWDGE rings to halve the first-matmul data latency.
  - HAM prewarm: dummy matmul burst flips the PE clock gate to 2.4GHz
    before the first real data lands.
"""

import numpy as np

B, S, ENC, DEC = 64, 2048, 512, 512
NCORES = 8
BL = B // NCORES          # batches per core
P = 128
EC = ENC // P             # 4 e-chunks
DC = DEC // P             # 4 d-chunks
ST = 512                  # matmul moving free-dim tile / DMA block
NST = S // ST             # 4 s-blocks per batch
HT = 1024                 # psum energy tile free size (one half)
NH = S // HT              # 2 halves
NSC = S // P              # 16 s-chunks for the weighted sum
HSC = HT // P             # 8 s-chunks per half
NWARM = 56                # prewarm matmuls
FP8 = True                # energy matmul in fp8e4m3 with DoubleRow
G = 2                     # 256-wide DoubleRow contraction chunks
KO = 2                    # k-tiles per DoubleRow matmul

_PROGRAM = None


def _build_program():
    import concourse.mybir as mybir
    import concourse.tile as tile
    from concourse import bacc
    from contextlib import ExitStack

    fp32 = mybir.dt.float32
    bf16 = mybir.dt.bfloat16
    fp8 = mybir.dt.float8e4
    AF = mybir.ActivationFunctionType
    ALU = mybir.AluOpType
    edt = fp8 if FP8 else bf16

    nc = bacc.Bacc("TRN2", debug=False, target_bir_lowering=False,
                   num_devices=NCORES)

    if FP8:
        enc4_d = nc.dram_tensor("encT4", [BL, NST, P, G, KO, ST], fp8,
                                kind="ExternalInput").ap()
        we_d = nc.dram_tensor("weT", [G, KO, P, DEC], fp8,
                              kind="ExternalInput").ap()
    else:
        enc4_d = nc.dram_tensor("encT4", [BL, NST, P, EC, ST], bf16,
                                kind="ExternalInput").ap()
        we_d = nc.dram_tensor("weT", [EC, P, DEC], bf16,
                              kind="ExternalInput").ap()
    encn_d = nc.dram_tensor("encN", [BL, NH, P, HSC, ENC], bf16,
                            kind="ExternalInput").ap()
    biasT_d = nc.dram_tensor("biasT", [P, DC, BL], fp32,
                             kind="ExternalInput").ap()
    wv_d = nc.dram_tensor("wv", [P, DC], fp32, kind="ExternalInput").ap()
    out_d = nc.dram_tensor("out", [BL, ENC], fp32, kind="ExternalOutput").ap()

    with tile.TileContext(nc) as tc, ExitStack() as ctx:
        const = ctx.enter_context(tc.tile_pool(name="const", bufs=1))
        # 3 energy-psum bufs (6 banks) decouple the matmul stream from
        # tanh completions; scth+orow share the remaining 2 banks, with
        # the softmax-sum row reusing the retired scth tile
        ps_e = ctx.enter_context(tc.tile_pool(name="ps_e", bufs=3, space="PSUM"))
        ps_sc = ctx.enter_context(tc.tile_pool(name="ps_sc", bufs=2, space="PSUM"))
        enc4_pool = ctx.enter_context(tc.tile_pool(name="enc4p", bufs=12))
        encn_pool = ctx.enter_context(tc.tile_pool(name="encnp", bufs=8))
        tanh_pool = ctx.enter_context(tc.tile_pool(name="tanhp", bufs=6))
        wve_pool = ctx.enter_context(tc.tile_pool(name="wvep", bufs=6))
        wvs_pool = ctx.enter_context(tc.tile_pool(name="wvsp", bufs=8))
        probs_pool = ctx.enter_context(tc.tile_pool(name="probsp", bufs=4))
        stage_pool = ctx.enter_context(tc.tile_pool(name="stagep", bufs=4))

        if FP8:
            we_sb = const.tile([P, G, KO, DEC], fp8)
        else:
            we_sb = const.tile([P, EC, DEC], bf16)
        biasT_sb = const.tile([P, DC, BL], fp32)
        wv_sb = const.tile([P, DC], fp32)
        ones_sb = const.tile([P, 1], bf16)
        warm_sb = const.tile([P, P], bf16)
        warmout_sb = const.tile([1, 1], fp32)
        ssum_sb = const.tile([1, BL], fp32)
        rs_sb = const.tile([1, BL], fp32)

        nc.vector.memset(warm_sb[:], 1.0)
        nc.vector.memset(ones_sb[:], 1.0)

        if FP8:
            nc.sync.dma_start(we_sb[:], we_d.rearrange("g k p d -> p g k d"))
        else:
            nc.sync.dma_start(we_sb[:], we_d.rearrange("e p d -> p e d"))
        nc.scalar.dma_start(biasT_sb[:], biasT_d)
        nc.scalar.dma_start(wv_sb[:], wv_d)

        # HAM prewarm: dummy accumulating matmuls, no data deps
        wps = ps_e.tile([P, P], fp32, tag="pse", name="warmps")
        for i in range(NWARM):
            nc.tensor.matmul(wps[:], lhsT=warm_sb[:], rhs=warm_sb[:],
                             start=(i == 0), stop=(i == NWARM - 1))
        nc.vector.tensor_copy(warmout_sb[:], wps[0:1, 0:1])

        enc4_t, encn_t = {}, {}
        probs_t, orow_t, scth_t = {}, {}, {}
        run_t = {}

        def issue_dmas(b):
            for st in range(NST):
                shape = [P, G, KO, ST] if FP8 else [P, EC, ST]
                t = enc4_pool.tile(shape, edt, tag="enc4",
                                   name=f"enc4_{b}_{st}")
                # batch 0: odd blocks via the scalar HWDGE ring so the
                # two FIFO chains deliver the first batch in parallel
                eng = nc.scalar if (b == 0 and st % 2 == 1) else nc.sync
                eng.dma_start(t[:], enc4_d[b, st])
                enc4_t[(b, st)] = t
            for h in range(NH):
                t = encn_pool.tile([P, HSC, ENC], bf16, tag="encn",
                                   name=f"encn{b}_{h}")
                nc.sync.dma_start(t[:], encn_d[b, h])
                encn_t[(b, h)] = t

        def emit_energy(b, h, dc):
            eps = ps_e.tile([P, HT], fp32, tag="pse", name=f"eps{b}_{h}_{dc}")
            for st in range(HT // ST):
                blk = h * (HT // ST) + st
                if FP8:
                    for g in range(G):
                        nc.tensor.matmul(
                            eps[:, st * ST:(st + 1) * ST],
                            lhsT=we_sb[:, g, :, dc * P:(dc + 1) * P],
                            rhs=enc4_t[(b, blk)][:, g, :, :],
                            start=(g == 0), stop=(g == G - 1),
                            perf_mode=mybir.MatmulPerfMode.DoubleRow)
                else:
                    for ec in range(EC):
                        nc.tensor.matmul(
                            eps[:, st * ST:(st + 1) * ST],
                            lhsT=we_sb[:, ec, dc * P:(dc + 1) * P],
                            rhs=enc4_t[(b, blk)][:, ec, :],
                            start=(ec == 0), stop=(ec == EC - 1))
            t = tanh_pool.tile([P, HT], bf16, tag="tanh",
                               name=f"tanh{b}_{h}_{dc}")
            nc.scalar.activation(t[:], eps[:], AF.Tanh,
                                 bias=biasT_sb[:, dc, b:b + 1])
            # wv multiply + running sum on DVE; the post-dc3 critical
            # path is one mul + one add
            wve_t = wve_pool.tile([P, HT], bf16, tag="wve",
                                  name=f"wve{b}_{h}_{dc}")
            nc.vector.tensor_scalar_mul(wve_t[:], t[:], wv_sb[:, dc:dc + 1])
            if dc == 0:
                run_t[(b, h)] = wve_t
            else:
                nxt = wvs_pool.tile([P, HT], bf16, tag="wvs",
                                    name=f"wvs{b}_{h}_{dc}")
                nc.vector.tensor_add(nxt[:], run_t[(b, h)][:], wve_t[:])
                run_t[(b, h)] = nxt

        def emit_scores(b, h, scth):
            asum = run_t.pop((b, h))
            for sci in range(HSC):
                nc.tensor.matmul(scth[:, sci:sci + 1],
                                 lhsT=asum[:, sci * P:(sci + 1) * P],
                                 rhs=ones_sb[:], start=True, stop=True)

        def emit_wsum(pb, ph, lo, hi):
            if ph == 0 and lo == 0:
                orow_t[pb] = ps_sc.tile([1, ENC], fp32, tag="sc",
                                        name=f"orow{pb}")
            orow = orow_t[pb]
            if ph == NH - 1 and lo == 0:
                # softmax normalizer: emitted before the final burst so
                # recip runs on DVE while the orow matmuls stream; the
                # [1, 16] row reuses the retired scth tile of (pb, 1)
                sums_ps = scth_t[(pb, NH - 1)][0:1, 0:NSC]
                nc.tensor.matmul(sums_ps, lhsT=ones_sb[:],
                                 rhs=probs_t[pb][:], start=True, stop=True)
                nc.vector.tensor_reduce(ssum_sb[0:1, pb:pb + 1], sums_ps,
                                        axis=mybir.AxisListType.X, op=ALU.add)
                nc.vector.reciprocal(rs_sb[0:1, pb:pb + 1],
                                     ssum_sb[0:1, pb:pb + 1])
            for c in range(lo, hi):
                cc = ph * HSC + c
                nc.tensor.matmul(
                    orow[:], lhsT=probs_t[pb][:, cc:cc + 1],
                    rhs=encn_t[(pb, ph)][:, c, :],
                    start=(cc == 0), stop=(cc == NSC - 1))
            if ph == NH - 1 and hi == HSC:
                ostg = stage_pool.tile([1, ENC], fp32, tag="stg",
                                       name=f"ostg{pb}")
                nc.vector.tensor_scalar_mul(ostg[:], orow[:],
                                            rs_sb[0:1, pb:pb + 1])
                nc.scalar.dma_start(out_d[pb:pb + 1, :], ostg[:])

        def emit_e0(b, h):
            # first energy chunk of a half; hoisting it across the half
            # boundary requires the batch's DMAs/probsT to exist
            if h == 0:
                issue_dmas(b)
                probs_t[b] = probs_pool.tile([P, NSC], bf16, tag="probst",
                                             name=f"probsT{b}")
            emit_energy(b, h, 0)

        halves = [(b, h) for b in range(BL) for h in range(NH)]
        pend = None
        emit_e0(*halves[0])
        for i, (b, h) in enumerate(halves):
            emit_energy(b, h, 1)
            emit_energy(b, h, 2)
            emit_energy(b, h, 3)
            # previous half's weighted-sum burst, then the NEXT half's
            # first energy chunk: both dependency-free PE filler that
            # covers this half's tanh3/mul/add chain ahead of the score
            # matmuls
            if pend is not None:
                emit_wsum(*pend, 0, HSC)
            if i + 1 < len(halves):
                emit_e0(*halves[i + 1])
            scth = ps_sc.tile([P, NSC], fp32, tag="sc",
                              name=f"scth{b}_{h}")
            scth_t[(b, h)] = scth
            emit_scores(b, h, scth)
            nc.scalar.activation(probs_t[b][:, h * HSC:(h + 1) * HSC],
                                 scth[:, 0:HSC], AF.Exp)
            pend = (b, h)

        # keep the PE clock-gate warm through the final exp wait so the
        # last weighted-sum burst runs at full clock
        wps2 = ps_e.tile([P, P], fp32, tag="pse", name="warmps2")
        for i in range(24):
            nc.tensor.matmul(wps2[:], lhsT=warm_sb[:], rhs=warm_sb[:],
                             start=(i == 0), stop=(i == 23))
        nc.vector.tensor_copy(warmout_sb[:], wps2[0:1, 0:1])
        emit_wsum(*pend, 0, HSC)

    nc.compile()
    return nc


def _get_program():
    global _PROGRAM
    if _PROGRAM is None:
        _PROGRAM = _build_program()
    return _PROGRAM


def _make_in_maps(hidden, encoder_outputs, W_attn, b_attn, w_v):
    import ml_dtypes
    bf = ml_dtypes.bfloat16
    f8 = ml_dtypes.float8_e4m3fn
    W_h, W_e = W_attn[:DEC], W_attn[DEC:]
    if FP8:
        # [G, KO, P, DEC]: contraction index e = g*256 + ko*128 + ki
        weT = np.ascontiguousarray(
            np.asarray(W_e).reshape(G, KO, P, DEC).astype(f8))
    else:
        weT = np.ascontiguousarray(
            np.asarray(W_e).reshape(EC, P, DEC).astype(bf))
    wv = np.ascontiguousarray(np.asarray(w_v, np.float32).reshape(DC, P).T)
    # h_proj host-side: [B, DEC]
    hproj = (np.asarray(hidden, np.float32) @ np.asarray(W_h, np.float32)
             + np.asarray(b_attn, np.float32))
    in_maps = []
    for c in range(NCORES):
        eb = np.asarray(encoder_outputs[c * BL:(c + 1) * BL])
        if FP8:
            # [BL, NST, P, G, KO, ST]: e = g*256 + ko*128 + p, one
            # contiguous 2KB row per partition per block
            enc4 = np.ascontiguousarray(
                eb.transpose(0, 2, 1).reshape(BL, G, KO, P, NST, ST)
                .transpose(0, 4, 3, 1, 2, 5).astype(f8))
        else:
            enc4 = np.ascontiguousarray(
                eb.transpose(0, 2, 1).reshape(BL, EC, P, NST, ST)
                .transpose(0, 3, 2, 1, 4).astype(bf))
        # [BL, NH, P, HSC, ENC]: partition p gathers s = h*HT + c*P + p
        encN = np.ascontiguousarray(
            eb.reshape(BL, NH, HSC, P, ENC).transpose(0, 1, 3, 2, 4)
            .astype(bf))
        hp = hproj[c * BL:(c + 1) * BL]          # [BL, DEC]
        biasT = np.ascontiguousarray(
            hp.T.reshape(DC, P, BL).transpose(1, 0, 2))  # [P, DC, BL]
        in_maps.append({"encT4": enc4, "encN": encN, "weT": weT,
                        "biasT": biasT, "wv": wv})
    return in_maps


def _install_trace_hooks():
    """The agent image's antenv lacks axon_hooks; recreate it from the
    ctypes NTFF profile shim in trn_agent_boot, and stub the fish-bucket
    artifact upload so the trace path stays local."""
    import sys, types
    if "antenv.axon_hooks" not in sys.modules:
        mod = types.ModuleType("antenv.axon_hooks")
        mod._hook = None
        mod.set_axon_ntff_profile_hook = lambda h: setattr(mod, "_hook", h)
        mod.get_axon_ntff_profile_hook = lambda: mod._hook
        sys.modules["antenv.axon_hooks"] = mod
        import antenv
        antenv.axon_hooks = mod
        try:
            from trn_agent_boot.trn_boot import _ntff_profile_via_ctypes
            mod._hook = _ntff_profile_via_ctypes("/opt/axon/libaxon_pjrt.so")
        except Exception as e:
            print(f"NTFF hook install failed: {e}")
    import concourse.bass_utils as bu
    bu.upload_artifacts = lambda tmpdir: f"local:{tmpdir}"


def run(hidden, encoder_outputs, W_attn, b_attn, w_v, trace=False, tmpdir=None):
    from concourse.bass_utils import run_bass_kernel_spmd
    if trace:
        _install_trace_hooks()
    nc = _get_program()
    in_maps = _make_in_maps(hidden, encoder_outputs, W_attn, b_attn, w_v)
    res = run_bass_kernel_spmd(nc, in_maps, list(range(NCORES)),
                               trace=trace, tmpdir=tmpdir)
    out = np.concatenate([np.asarray(res.results[c]["out"], np.float32)
                          for c in range(NCORES)], axis=0)
    return out, res


def kernel(hidden, encoder_outputs, W_attn, b_attn, w_v):
    out, _ = run(hidden, encoder_outputs, W_attn, b_attn, w_v)
    return out


# revision 60
# speedup vs baseline: 1.0009x; 1.0009x over previous
"""Bahdanau-attention Trainium2 kernel (data-parallel over 8 NeuronCores).

Computation (per batch row b):
    energy[s, d] = tanh(hidden[b] @ W_h + enc[b, s] @ W_e + b_attn)   [S, D]
    scores[s]    = energy[s] . w_v                                     [S]
    attn         = softmax(scores)                                     [S]
    out[b]       = sum_s attn[s] * enc[b, s]                           [E]

v10 — fp8 DoubleRow energy, v8 score structure, tuned pipeline:
  - h_proj precomputed host-side; biasT [P, DC, BL] fp32 input.
  - energy matmuls in fp8e4m3 with perf_mode=DoubleRow (K=256 per MM):
    enc staged [BL, NST, P, G, KO, ST], W_e [G, KO, P, DEC].
  - scores: tanh -> x w_v (DVE tensor_scalar) -> running-sum adds (DVE,
    keeps the post-dc3 chain to one add) -> 8 single-shot column
    matmuls (asum slice stationary x ones) -> exp into probsT.
  - per-half emission order spreads the previous half's weighted-sum
    burst (dependency-free PE filler) through the energy stream:
      E0 E1 W[0:2] E2 W[2:4] E3 W[4:8] S exp
    covering both the psum-recycle waits (E_i needs tanh_{i-2} done,
    ps_e has 2 bufs) and the tanh3->mul->add chain ahead of S.
  - softmax sums/recip emitted BEFORE the final weighted-sum burst so
    the output scale is ready when the orow accumulation stops; output
    row scaling on DVE (tensor_scalar), off the ACT engine.
  - single-dma_start bulk transfers (~600ns issue each), 2-8KB
    contiguous per partition row; batch 0's encT blocks split across
    both HWDGE rings to halve the first-matmul data latency.
  - HAM prewarm: dummy matmul burst flips the PE clock gate to 2.4GHz
    before the first real data lands.
"""

import numpy as np

B, S, ENC, DEC = 64, 2048, 512, 512
NCORES = 8
BL = B // NCORES          # batches per core
P = 128
EC = ENC // P             # 4 e-chunks
DC = DEC // P             # 4 d-chunks
ST = 512                  # matmul moving free-dim tile / DMA block
NST = S // ST             # 4 s-blocks per batch
HT = 1024                 # psum energy tile free size (one half)
NH = S // HT              # 2 halves
NSC = S // P              # 16 s-chunks for the weighted sum
HSC = HT // P             # 8 s-chunks per half
NWARM = 56                # prewarm matmuls
FP8 = True                # energy matmul in fp8e4m3 with DoubleRow
G = 2                     # 256-wide DoubleRow contraction chunks
KO = 2                    # k-tiles per DoubleRow matmul

_PROGRAM = None


def _build_program():
    import concourse.mybir as mybir
    import concourse.tile as tile
    from concourse import bacc
    from contextlib import ExitStack

    fp32 = mybir.dt.float32
    bf16 = mybir.dt.bfloat16
    fp8 = mybir.dt.float8e4
    AF = mybir.ActivationFunctionType
    ALU = mybir.AluOpType
    edt = fp8 if FP8 else bf16

    nc = bacc.Bacc("TRN2", debug=False, target_bir_lowering=False,
                   num_devices=NCORES)

    if FP8:
        enc4_d = nc.dram_tensor("encT4", [BL, NST, P, G, KO, ST], fp8,
                                kind="ExternalInput").ap()
        we_d = nc.dram_tensor("weT", [G, KO, P, DEC], fp8,
                              kind="ExternalInput").ap()
    else:
        enc4_d = nc.dram_tensor("encT4", [BL, NST, P, EC, ST], bf16,
                                kind="ExternalInput").ap()
        we_d = nc.dram_tensor("weT", [EC, P, DEC], bf16,
                              kind="ExternalInput").ap()
    encn_d = nc.dram_tensor("encN", [BL, NH, P, HSC, ENC], bf16,
                            kind="ExternalInput").ap()
    biasT_d = nc.dram_tensor("biasT", [P, DC, BL], fp32,
                             kind="ExternalInput").ap()
    wv_d = nc.dram_tensor("wv", [P, DC], fp32, kind="ExternalInput").ap()
    out_d = nc.dram_tensor("out", [BL, ENC], fp32, kind="ExternalOutput").ap()

    with tile.TileContext(nc) as tc, ExitStack() as ctx:
        const = ctx.enter_context(tc.tile_pool(name="const", bufs=1))
        # 3 energy-psum bufs (6 banks) decouple the matmul stream from
        # tanh completions; scth+orow share the remaining 2 banks, with
        # the softmax-sum row reusing the retired scth tile
        ps_e = ctx.enter_context(tc.tile_pool(name="ps_e", bufs=3, space="PSUM"))
        ps_sc = ctx.enter_context(tc.tile_pool(name="ps_sc", bufs=2, space="PSUM"))
        enc4_pool = ctx.enter_context(tc.tile_pool(name="enc4p", bufs=12))
        encn_pool = ctx.enter_context(tc.tile_pool(name="encnp", bufs=8))
        tanh_pool = ctx.enter_context(tc.tile_pool(name="tanhp", bufs=6))
        wve_pool = ctx.enter_context(tc.tile_pool(name="wvep", bufs=6))
        wvs_pool = ctx.enter_context(tc.tile_pool(name="wvsp", bufs=8))
        probs_pool = ctx.enter_context(tc.tile_pool(name="probsp", bufs=4))
        stage_pool = ctx.enter_context(tc.tile_pool(name="stagep", bufs=4))

        if FP8:
            we_sb = const.tile([P, G, KO, DEC], fp8)
        else:
            we_sb = const.tile([P, EC, DEC], bf16)
        biasT_sb = const.tile([P, DC, BL], fp32)
        wv_sb = const.tile([P, DC], fp32)
        ones_sb = const.tile([P, 1], bf16)
        warm_sb = const.tile([P, P], bf16)
        warmout_sb = const.tile([1, 1], fp32)
        ssum_sb = const.tile([1, BL], fp32)
        rs_sb = const.tile([1, BL], fp32)

        nc.vector.memset(warm_sb[:], 1.0)
        nc.vector.memset(ones_sb[:], 1.0)

        if FP8:
            nc.sync.dma_start(we_sb[:], we_d.rearrange("g k p d -> p g k d"))
        else:
            nc.sync.dma_start(we_sb[:], we_d.rearrange("e p d -> p e d"))
        nc.scalar.dma_start(biasT_sb[:], biasT_d)
        nc.scalar.dma_start(wv_sb[:], wv_d)

        # HAM prewarm: dummy accumulating matmuls, no data deps
        wps = ps_e.tile([P, P], fp32, tag="pse", name="warmps")
        for i in range(NWARM):
            nc.tensor.matmul(wps[:], lhsT=warm_sb[:], rhs=warm_sb[:],
                             start=(i == 0), stop=(i == NWARM - 1))
        nc.vector.tensor_copy(warmout_sb[:], wps[0:1, 0:1])

        enc4_t, encn_t = {}, {}
        probs_t, orow_t, scth_t = {}, {}, {}
        run_t = {}

        def issue_dmas(b):
            for st in range(NST):
                shape = [P, G, KO, ST] if FP8 else [P, EC, ST]
                t = enc4_pool.tile(shape, edt, tag="enc4",
                                   name=f"enc4_{b}_{st}")
                # batch 0: odd blocks via the scalar HWDGE ring so the
                # two FIFO chains deliver the first batch in parallel
                eng = nc.scalar if (b == 0 and st % 2 == 1) else nc.sync
                eng.dma_start(t[:], enc4_d[b, st])
                enc4_t[(b, st)] = t
            for h in range(NH):
                t = encn_pool.tile([P, HSC, ENC], bf16, tag="encn",
                                   name=f"encn{b}_{h}")
                nc.sync.dma_start(t[:], encn_d[b, h])
                encn_t[(b, h)] = t

        def emit_energy(b, h, dc):
            eps = ps_e.tile([P, HT], fp32, tag="pse", name=f"eps{b}_{h}_{dc}")
            for st in range(HT // ST):
                blk = h * (HT // ST) + st
                if FP8:
                    for g in range(G):
                        nc.tensor.matmul(
                            eps[:, st * ST:(st + 1) * ST],
                            lhsT=we_sb[:, g, :, dc * P:(dc + 1) * P],
                            rhs=enc4_t[(b, blk)][:, g, :, :],
                            start=(g == 0), stop=(g == G - 1),
                            perf_mode=mybir.MatmulPerfMode.DoubleRow)
                else:
                    for ec in range(EC):
                        nc.tensor.matmul(
                            eps[:, st * ST:(st + 1) * ST],
                            lhsT=we_sb[:, ec, dc * P:(dc + 1) * P],
                            rhs=enc4_t[(b, blk)][:, ec, :],
                            start=(ec == 0), stop=(ec == EC - 1))
            t = tanh_pool.tile([P, HT], bf16, tag="tanh",
                               name=f"tanh{b}_{h}_{dc}")
            nc.scalar.activation(t[:], eps[:], AF.Tanh,
                                 bias=biasT_sb[:, dc, b:b + 1])
            # wv multiply + running sum on DVE; the post-dc3 critical
            # path is one mul + one add
            wve_t = wve_pool.tile([P, HT], bf16, tag="wve",
                                  name=f"wve{b}_{h}_{dc}")
            nc.vector.tensor_scalar_mul(wve_t[:], t[:], wv_sb[:, dc:dc + 1])
            if dc == 0:
                run_t[(b, h)] = wve_t
            else:
                nxt = wvs_pool.tile([P, HT], bf16, tag="wvs",
                                    name=f"wvs{b}_{h}_{dc}")
                nc.vector.tensor_add(nxt[:], run_t[(b, h)][:], wve_t[:])
                run_t[(b, h)] = nxt

        def emit_scores(b, h, scth):
            asum = run_t.pop((b, h))
            for sci in range(HSC):
                nc.tensor.matmul(scth[:, sci:sci + 1],
                                 lhsT=asum[:, sci * P:(sci + 1) * P],
                                 rhs=ones_sb[:], start=True, stop=True)

        def emit_wsum(pb, ph, lo, hi):
            if ph == 0 and lo == 0:
                orow_t[pb] = ps_sc.tile([1, ENC], fp32, tag="sc",
                                        name=f"orow{pb}")
            orow = orow_t[pb]
            if ph == NH - 1 and lo == 0:
                # softmax normalizer: emitted before the final burst so
                # recip runs on DVE while the orow matmuls stream; the
                # [1, 16] row reuses the retired scth tile of (pb, 1)
                sums_ps = scth_t[(pb, NH - 1)][0:1, 0:NSC]
                nc.tensor.matmul(sums_ps, lhsT=ones_sb[:],
                                 rhs=probs_t[pb][:], start=True, stop=True)
                nc.vector.tensor_reduce(ssum_sb[0:1, pb:pb + 1], sums_ps,
                                        axis=mybir.AxisListType.X, op=ALU.add)
                nc.vector.reciprocal(rs_sb[0:1, pb:pb + 1],
                                     ssum_sb[0:1, pb:pb + 1])
            for c in range(lo, hi):
                cc = ph * HSC + c
                nc.tensor.matmul(
                    orow[:], lhsT=probs_t[pb][:, cc:cc + 1],
                    rhs=encn_t[(pb, ph)][:, c, :],
                    start=(cc == 0), stop=(cc == NSC - 1))
            if ph == NH - 1 and hi == HSC:
                ostg = stage_pool.tile([1, ENC], fp32, tag="stg",
                                       name=f"ostg{pb}")
                nc.vector.tensor_scalar_mul(ostg[:], orow[:],
                                            rs_sb[0:1, pb:pb + 1])
                nc.scalar.dma_start(out_d[pb:pb + 1, :], ostg[:])

        def emit_e0(b, h):
            # first energy chunk of a half; hoisting it across the half
            # boundary requires the batch's DMAs/probsT to exist
            if h == 0:
                issue_dmas(b)
                probs_t[b] = probs_pool.tile([P, NSC], bf16, tag="probst",
                                             name=f"probsT{b}")
            emit_energy(b, h, 0)

        halves = [(b, h) for b in range(BL) for h in range(NH)]
        pend = None
        emit_e0(*halves[0])
        for i, (b, h) in enumerate(halves):
            emit_energy(b, h, 1)
            emit_energy(b, h, 2)
            emit_energy(b, h, 3)
            # previous half's weighted-sum burst, then the NEXT half's
            # first energy chunk: both dependency-free PE filler that
            # covers this half's tanh3/mul/add chain ahead of the score
            # matmuls
            if pend is not None:
                emit_wsum(*pend, 0, HSC)
            if i + 1 < len(halves):
                emit_e0(*halves[i + 1])
            scth = ps_sc.tile([P, NSC], fp32, tag="sc",
                              name=f"scth{b}_{h}")
            scth_t[(b, h)] = scth
            emit_scores(b, h, scth)
            nc.scalar.activation(probs_t[b][:, h * HSC:(h + 1) * HSC],
                                 scth[:, 0:HSC], AF.Exp)
            pend = (b, h)

        # keep the PE clock-gate warm through the final exp wait so the
        # last weighted-sum burst runs at full clock
        wps2 = ps_e.tile([P, P], fp32, tag="pse", name="warmps2")
        for i in range(24):
            nc.tensor.matmul(wps2[:], lhsT=warm_sb[:], rhs=warm_sb[:],
                             start=(i == 0), stop=(i == 23))
        nc.vector.tensor_copy(warmout_sb[:], wps2[0:1, 0:1])
        emit_wsum(*pend, 0, HSC)

    nc.compile()
    return nc


def _get_program():
    global _PROGRAM
    if _PROGRAM is None:
        _PROGRAM = _build_program()
    return _PROGRAM


def _make_in_maps(hidden, encoder_outputs, W_attn, b_attn, w_v):
    import ml_dtypes
    bf = ml_dtypes.bfloat16
    f8 = ml_dtypes.float8_e4m3fn
    W_h, W_e = W_attn[:DEC], W_attn[DEC:]
    if FP8:
        # [G, KO, P, DEC]: contraction index e = g*256 + ko*128 + ki
        weT = np.ascontiguousarray(
            np.asarray(W_e).reshape(G, KO, P, DEC).astype(f8))
    else:
        weT = np.ascontiguousarray(
            np.asarray(W_e).reshape(EC, P, DEC).astype(bf))
    wv = np.ascontiguousarray(np.asarray(w_v, np.float32).reshape(DC, P).T)
    # h_proj host-side: [B, DEC]
    hproj = (np.asarray(hidden, np.float32) @ np.asarray(W_h, np.float32)
             + np.asarray(b_attn, np.float32))
    in_maps = []
    for c in range(NCORES):
        eb = np.asarray(encoder_outputs[c * BL:(c + 1) * BL])
        if FP8:
            # [BL, NST, P, G, KO, ST]: e = g*256 + ko*128 + p, one
            # contiguous 2KB row per partition per block
            enc4 = np.ascontiguousarray(
                eb.transpose(0, 2, 1).reshape(BL, G, KO, P, NST, ST)
                .transpose(0, 4, 3, 1, 2, 5).astype(f8))
        else:
            enc4 = np.ascontiguousarray(
                eb.transpose(0, 2, 1).reshape(BL, EC, P, NST, ST)
                .transpose(0, 3, 2, 1, 4).astype(bf))
        # [BL, NH, P, HSC, ENC]: partition p gathers s = h*HT + c*P + p
        encN = np.ascontiguousarray(
            eb.reshape(BL, NH, HSC, P, ENC).transpose(0, 1, 3, 2, 4)
            .astype(bf))
        hp = hproj[c * BL:(c + 1) * BL]          # [BL, DEC]
        biasT = np.ascontiguousarray(
            hp.T.reshape(DC, P, BL).transpose(1, 0, 2))  # [P, DC, BL]
        in_maps.append({"encT4": enc4, "encN": encN, "weT": weT,
                        "biasT": biasT, "wv": wv})
    return in_maps


def _install_trace_hooks():
    """The agent image's antenv lacks axon_hooks; recreate it from the
    ctypes NTFF profile shim in trn_agent_boot, and stub the fish-bucket
    artifact upload so the trace path stays local."""
    import sys, types
    if "antenv.axon_hooks" not in sys.modules:
        mod = types.ModuleType("antenv.axon_hooks")
        mod._hook = None
        mod.set_axon_ntff_profile_hook = lambda h: setattr(mod, "_hook", h)
        mod.get_axon_ntff_profile_hook = lambda: mod._hook
        sys.modules["antenv.axon_hooks"] = mod
        import antenv
        antenv.axon_hooks = mod
        try:
            from trn_agent_boot.trn_boot import _ntff_profile_via_ctypes
            mod._hook = _ntff_profile_via_ctypes("/opt/axon/libaxon_pjrt.so")
        except Exception as e:
            print(f"NTFF hook install failed: {e}")
    import concourse.bass_utils as bu
    bu.upload_artifacts = lambda tmpdir: f"local:{tmpdir}"


def run(hidden, encoder_outputs, W_attn, b_attn, w_v, trace=False, tmpdir=None):
    from concourse.bass_utils import run_bass_kernel_spmd
    if trace:
        _install_trace_hooks()
    nc = _get_program()
    in_maps = _make_in_maps(hidden, encoder_outputs, W_attn, b_attn, w_v)
    res = run_bass_kernel_spmd(nc, in_maps, list(range(NCORES)),
                               trace=trace, tmpdir=tmpdir)
    out = np.concatenate([np.asarray(res.results[c]["out"], np.float32)
                          for c in range(NCORES)], axis=0)
    return out, res


def kernel(hidden, encoder_outputs, W_attn, b_attn, w_v):
    out, _ = run(hidden, encoder_outputs, W_attn, b_attn, w_v)
    return out
